# revision 1
# baseline (speedup 1.0000x reference)
"""Bass/Trainium2 kernel for the GCL loss function (nn_GCL_46076409151702).

Math (reference):
    g_s = segment_sum(z_s, batch_s, 512)            s in {1,2}
    zn_s, gn_s = l2norm rows
    pos11 = <zn1[i], gn1[b1[i]]>, cross12 = <zn1[i], gn2[b1[i]]>
    pos22 = <zn2[i], gn2[b2[i]]>, cross21 = <zn2[i], gn1[b2[i]]>
    d_s = softplus(-cross) - softplus(-pos)
    out  = sqrt(sum d1^2) + sqrt(sum d2^2)

Strategy: data-parallel over nodes on 8 cores; AllReduce the per-graph sums.
Key data property: batch ids are *sorted*, and every graph has >=128 nodes,
so each 128-node tile touches at most 2 consecutive graphs {A_t, A_t+1}.
All index-dependent structures (per-tile 2-column masks, scatter matrix,
gather indices) are built host-side from batch and passed as input tensors,
so the compiled graph is input-independent.

Per-core device pipeline:
  P1  per 128-node tile: seg-matmul (lhsT=z_bf16, rhs=mask2 -> [d,2]
      contribs, accumulated region-wise into one PSUM bank per side),
      ACT square+accum (row norms)
  P2  scatter contribs -> g[512,128] via one-hot matmuls; AllReduce g1|g2;
      zT loaded via hardware DMA-transpose (overlaps the collective)
  P3  normalize g rows; P4 transpose to gT layout; P5 gather per-tile
      candidate columns (gpsimd indirect_copy)
  P6  per tile: matvec (lhsT=zT bf16, rhs=4 candidate cols) -> 4 dots/node,
      region-wise into 2 PSUM banks
  P7  select by mask, scale by row rsqrt, softplus, accumulate d^2; output
      per-core partial [2] -> host: sqrt(sum) + sqrt(sum)
"""

import numpy as np
import ml_dtypes

import concourse.bass as bass
import concourse.bacc as bacc
import concourse.mybir as mybir
import concourse.tile as tile
from concourse.bass_utils import run_bass_kernel_spmd
from concourse.masks import make_identity

F32 = mybir.dt.float32
BF16 = mybir.dt.bfloat16
U16 = mybir.dt.uint16
AL = mybir.AluOpType
AF = mybir.ActivationFunctionType

NCORES = 8
G = 512          # num graphs
D = 128          # feature dim
P = 128          # partitions
NT = 98          # tiles per core
R = NT * P       # rows per core = 12544
NPAD = NCORES * R  # 100352
CK = 2           # candidate graphs per tile (sorted batch, counts >= 128)
GW = G + 4       # per-side column width in gT table (pad col for A+1==512)
NIDX = NT * 2 * CK * 2  # gather cols: NT tiles x 2 sides x (CK pos + CK cross)
EPS = 1e-12
CH = 49          # tiles per z DMA group (1.57 MB per DMA)
NGRP = NT // CH  # 2
HT = 49          # tiles per P6 psum accumulator bank (49*8 f32 = 1568B)


def build_nc(finalize=True, stage=99):
    # stage (debug bisect): 0=inputs only, 1=P1, 2=+collective, 3=+normalize,
    # 4=+gather, 99=full.
    # Bacc (not plain Bass): its compile pipeline legalizes sync waits for
    # TRN2's one-wait-per-instruction constraint and allocates registers.
    nc = bacc.Bacc(None, target_bir_lowering=False, debug=False)
    z1 = nc.dram_tensor("z1", [R, D], BF16, kind="ExternalInput")
    z2 = nc.dram_tensor("z2", [R, D], BF16, kind="ExternalInput")
    # interleaved per-tile masks: mab[p, CK*t+j] = (batch[t*128+p] == A_t + j)
    mab1 = nc.dram_tensor("mab1", [P, NT * CK], BF16, kind="ExternalInput")
    mab2 = nc.dram_tensor("mab2", [P, NT * CK], BF16, kind="ExternalInput")
    # scatter one-hot: sel[c, g] = 1 iff A_{c//CK} + c%CK == g (c < NT*CK)
    sel1 = nc.dram_tensor("sel1", [2 * P, G], F32, kind="ExternalInput")
    sel2 = nc.dram_tensor("sel2", [2 * P, G], F32, kind="ExternalInput")
    # indirect_copy wrapped indices into the [P, 2*GW] gT table
    gidx = nc.dram_tensor("gidx", [P, NIDX // 16], U16, kind="ExternalInput")
    out_part = nc.dram_tensor("out_part", [2, 1], F32, kind="ExternalOutput")

    zs = [z1, z2]
    mabs = [mab1, mab2]
    sels = [sel1, sel2]

    def _body(tc):
        with (
            tc.tile_pool(name="const", bufs=1) as constp,
            tc.tile_pool(name="stash", bufs=1) as stashp,
            tc.tile_pool(name="zin", bufs=2) as zinp,
            tc.tile_pool(name="scr", bufs=4) as scrp,
            tc.tile_pool(name="small", bufs=4) as smallp,
            tc.tile_pool(name="pstag", bufs=1, space="PSUM") as pstagp,
            tc.tile_pool(name="pcand", bufs=1, space="PSUM") as pcandp,
            tc.tile_pool(name="pzt", bufs=3, space="PSUM") as pztp,
            tc.tile_pool(name="dram", bufs=1, space="DRAM") as dramp,
        ):
            # ---- constants / inputs to SBUF ----
            ident = constp.tile([P, P], F32)
            make_identity(nc, ident[:])
            ones_col = constp.tile([P, 1], F32)
            nc.vector.memset(ones_col[:], 1.0)
            ones_bf = constp.tile([P, 1], BF16)
            nc.vector.memset(ones_bf[:], 1.0)

            mab_sb = []
            sel_sb = []
            for s in range(2):
                m = constp.tile([P, NT * CK], BF16, name=f"mab_sb{s}")
                nc.sync.dma_start(out=m[:], in_=mabs[s][:])
                mab_sb.append(m)
                s0 = constp.tile([P, G], F32, name=f"sel_sb{s}a")
                s1 = constp.tile([P, G], F32, name=f"sel_sb{s}b")
                nc.sync.dma_start(out=s0[:], in_=sels[s][0:P, :])
                nc.sync.dma_start(out=s1[:], in_=sels[s][P : 2 * P, :])
                sel_sb.append((s0, s1))
            gidx_sb = constp.tile([P, NIDX // 16], U16)
            nc.sync.dma_start(out=gidx_sb[:], in_=gidx[:])

            if stage <= 0:
                osb0 = smallp.tile([2, 1], F32, tag="osb")
                nc.vector.tensor_copy(out=osb0[:], in_=ones_col[0:2, 0:1])
                nc.sync.dma_start(out=out_part[:], in_=osb0[:])
                return

            # ---- persistent stashes ----
            zst = [stashp.tile([P, NT * P], BF16, name=f"zst{s}") for s in range(2)]
            stagT = [stashp.tile([P, NT * CK], F32, name=f"stagT{s}") for s in range(2)]
            ssq = [stashp.tile([P, NT], F32, name=f"ssq{s}") for s in range(2)]
            # cand8: per tile 8 cols = [s0:posA,posB,crossA,crossB | s1:...]
            cand8 = stashp.tile([P, NT * 8], F32, name="cand8")
            sqscr = stashp.tile([P, NT * P], BF16, name="sqscr")

            pstag = [
                pstagp.tile([P, NT * CK], F32, name=f"pstag{s}") for s in range(2)
            ]

            # ---- P1: stream z tiles; seg matmuls into region-wise PSUM ----
            for g in range(NGRP):
                for s in range(2):
                    zg = zinp.tile([P, CH * P], BF16, tag=f"zg{s}")
                    zr = zs[s].rearrange("(a k p) d -> a p k d", a=NGRP, k=CH, p=P)
                    nc.sync.dma_start(
                        out=zg[:].rearrange("p (k d) -> p k d", k=CH), in_=zr[g]
                    )
                    for k in range(CH):
                        t = g * CH + k
                        zt = zg[:, k * P : (k + 1) * P]
                        nc.tensor.matmul(
                            out=pstag[s][:, CK * t : CK * (t + 1)],
                            lhsT=zt,
                            rhs=mab_sb[s][:, CK * t : CK * (t + 1)],
                            start=True,
                            stop=True,
                        )
                        nc.scalar.activation(
                            out=sqscr[:, t * P : (t + 1) * P],
                            in_=zt,
                            func=AF.Square,
                            accum_out=ssq[s][:, t : t + 1],
                        )
            for s in range(2):
                nc.vector.tensor_copy(out=stagT[s][:], in_=pstag[s][:])

            if stage <= 1:
                osb1 = smallp.tile([2, 1], F32, tag="osb")
                nc.vector.tensor_copy(out=osb1[:], in_=stagT[0][0:2, 0:1])
                nc.sync.dma_start(out=out_part[:], in_=osb1[:])
                return

            # ---- P2: scatter contribs to g and AllReduce ----
            gloc = []
            for s in range(2):
                # staging [c, d] = transpose of stagT
                stg_a = smallp.tile([P, P], F32, tag="stg", bufs=2)
                stg_b = smallp.tile([P, P], F32, tag="stg", bufs=2)
                pta = pztp.tile([P, P], F32, tag="pzt")
                nc.tensor.transpose(out=pta[:], in_=stagT[s][:, 0:P], identity=ident[:])
                nc.vector.tensor_copy(out=stg_a[:], in_=pta[:])
                rem = NT * CK - P  # 68
                ptb = pztp.tile([P, P], F32, tag="pzt")
                nc.tensor.transpose(
                    out=ptb[:rem, :], in_=stagT[s][:, P : NT * CK], identity=ident[:]
                )
                nc.vector.tensor_copy(out=stg_b[:rem, :], in_=ptb[:rem, :])

                gl = stashp.tile([P, 4 * P], F32, name=f"gloc{s}")
                for gc in range(4):
                    pg = pztp.tile([P, P], F32, tag="pzt")
                    nc.tensor.matmul(
                        out=pg[:],
                        lhsT=sel_sb[s][0][:, gc * P : (gc + 1) * P],
                        rhs=stg_a[:],
                        start=True,
                        stop=False,
                    )
                    nc.tensor.matmul(
                        out=pg[:],
                        lhsT=sel_sb[s][1][:rem, gc * P : (gc + 1) * P],
                        rhs=stg_b[:rem, :],
                        start=False,
                        stop=True,
                    )
                    nc.vector.tensor_copy(out=gl[:, gc * P : (gc + 1) * P], in_=pg[:])
                gloc.append(gl)

            drin = dramp.tile([2, 4, P, P], F32)
            drout = dramp.tile([2, 4, P, P], F32, addr_space="Shared")
            for s in range(2):
                nc.sync.dma_start(
                    out=drin[s].rearrange("c p d -> p c d"),
                    in_=gloc[s][:].rearrange("p (c d) -> p c d", c=4),
                )
            nc.gpsimd.collective_compute(
                "AllReduce",
                AL.add,
                replica_groups=[list(range(NCORES))],
                ins=[drin.opt()],
                outs=[drout.opt()],
            )

            # zT stash via hardware DMA transpose (overlaps the collective)
            for s in range(2):
                nc.sync.dma_start_transpose(out=zst[s][:], in_=zs[s][:])

            if stage <= 2:
                g0 = smallp.tile([P, 4 * P], F32, tag="gsum", bufs=2)
                nc.sync.dma_start(
                    out=g0[:].rearrange("p (c d) -> p c d", c=4),
                    in_=drout[0].rearrange("c p d -> p c d"),
                )
                osb2 = smallp.tile([2, 1], F32, tag="osb")
                nc.vector.tensor_copy(out=osb2[:], in_=g0[0:2, 0:1])
                nc.sync.dma_start(out=out_part[:], in_=osb2[:])
                return

            # ---- P3/P4: normalize g rows, build gT table ----
            gtab = stashp.tile([P, 2 * GW], F32, name="gtab")
            nc.vector.memset(gtab[:], 0.0)
            for s in range(2):
                gsum = smallp.tile([P, 4 * P], F32, tag="gsum", bufs=2)
                nc.sync.dma_start(
                    out=gsum[:].rearrange("p (c d) -> p c d", c=4),
                    in_=drout[s].rearrange("c p d -> p c d"),
                )
                for gc in range(4):
                    chunk = gsum[:, gc * P : (gc + 1) * P]
                    sqg = scrp.tile([P, P], F32, tag="sq")
                    ss = smallp.tile([P, 1], F32, tag="nrm")
                    nc.scalar.activation(
                        out=sqg[:], in_=chunk, func=AF.Square, accum_out=ss[:]
                    )
                    nrm = smallp.tile([P, 1], F32, tag="nrm")
                    nc.scalar.activation(out=nrm[:], in_=ss[:], func=AF.Sqrt)
                    nc.vector.tensor_scalar(
                        out=nrm[:], in0=nrm[:], scalar1=EPS, scalar2=None, op0=AL.max
                    )
                    inv = smallp.tile([P, 1], F32, tag="nrm")
                    nc.vector.reciprocal(out=inv[:], in_=nrm[:])
                    nc.vector.tensor_scalar(
                        out=chunk,
                        in0=chunk,
                        scalar1=inv[:, 0:1],
                        scalar2=None,
                        op0=AL.mult,
                    )
                    pt = pztp.tile([P, P], F32, tag="pzt")
                    nc.tensor.transpose(out=pt[:], in_=chunk, identity=ident[:])
                    nc.vector.tensor_copy(
                        out=gtab[:, s * GW + gc * P : s * GW + (gc + 1) * P],
                        in_=pt[:],
                    )

            if stage <= 3:
                osb3 = smallp.tile([2, 1], F32, tag="osb")
                nc.vector.tensor_copy(out=osb3[:], in_=gtab[0:2, 0:1])
                nc.sync.dma_start(out=out_part[:], in_=osb3[:])
                return

            # ---- P5: gather candidate columns ----
            gsel_f = stashp.tile([P, NIDX], F32, name="gsel_f")
            nc.gpsimd.indirect_copy(gsel_f[:], gtab[:], gidx_sb[:], True)
            gsel = stashp.tile([P, NIDX], BF16, name="gsel")
            nc.vector.tensor_copy(out=gsel[:], in_=gsel_f[:])

            if stage <= 4:
                osb4 = smallp.tile([2, 1], F32, tag="osb")
                nc.vector.tensor_copy(out=osb4[:], in_=gsel[0:2, 0:1])
                nc.sync.dma_start(out=out_part[:], in_=osb4[:])
                return

            # ---- P6: per-tile matvecs, region-wise into 2 PSUM banks ----
            pcand = [
                pcandp.tile([P, HT * 8], F32, name=f"pcand{h}") for h in range(2)
            ]
            for t in range(NT):
                h, o = t // HT, (t % HT) * 8
                for s in range(2):
                    nc.tensor.matmul(
                        out=pcand[h][:, o + 4 * s : o + 4 * s + 4],
                        lhsT=zst[s][:, t * P : (t + 1) * P],
                        rhs=gsel[:, (t * 2 + s) * 4 : (t * 2 + s) * 4 + 4],
                        start=True,
                        stop=True,
                    )
            for h in range(2):
                nc.vector.tensor_copy(
                    out=cand8[:, h * HT * 8 : (h + 1) * HT * 8], in_=pcand[h][:]
                )

            # ---- P7: select, scale, softplus, reduce ----
            mabf = [stashp.tile([P, NT * CK], F32, name=f"mabf{s}") for s in range(2)]
            for s in range(2):
                nc.vector.tensor_copy(out=mabf[s][:], in_=mab_sb[s][:])
            d2col = smallp.tile([P, 2], F32, tag="d2col")
            for s in range(2):
                rn = smallp.tile([P, NT], F32, tag=f"fin{s}")
                nc.scalar.activation(out=rn[:], in_=ssq[s][:], func=AF.Sqrt)
                nc.vector.tensor_scalar(
                    out=rn[:], in0=rn[:], scalar1=EPS, scalar2=None, op0=AL.max
                )
                inv = smallp.tile([P, NT], F32, tag=f"fin{s}")
                nc.vector.reciprocal(out=inv[:], in_=rn[:])

                cv = cand8[:].rearrange("p (t w) -> p w t", w=8)
                mv = mabf[s][:].rearrange("p (t j) -> p j t", j=CK)
                quant = []
                for base in (4 * s, 4 * s + 2):  # pos cols, cross cols
                    ta = smallp.tile([P, NT], F32, tag=f"fin{s}")
                    tb = smallp.tile([P, NT], F32, tag=f"fin{s}")
                    nc.vector.tensor_tensor(
                        out=ta[:], in0=mv[:, 0, :], in1=cv[:, base, :], op=AL.mult
                    )
                    nc.vector.tensor_tensor(
                        out=tb[:], in0=mv[:, 1, :], in1=cv[:, base + 1, :], op=AL.mult
                    )
                    nc.vector.tensor_tensor(out=ta[:], in0=ta[:], in1=tb[:], op=AL.add)
                    # scale by per-node reciprocal norm, then
                    # softplus(-x) = -ln(sigmoid(x)); d uses the ln form
                    nc.vector.tensor_tensor(out=ta[:], in0=ta[:], in1=inv[:], op=AL.mult)
                    sg = smallp.tile([P, NT], F32, tag=f"fin{s}")
                    nc.scalar.activation(out=sg[:], in_=ta[:], func=AF.Sigmoid)
                    ll = smallp.tile([P, NT], F32, tag=f"fin{s}")
                    nc.scalar.activation(out=ll[:], in_=sg[:], func=AF.Ln)
                    quant.append(ll)
                # d = sp(-cross) - sp(-pos) = ln(sig(pos)) - ln(sig(cross))
                dd = smallp.tile([P, NT], F32, tag=f"fin{s}")
                nc.vector.tensor_tensor(
                    out=dd[:], in0=quant[0][:], in1=quant[1][:], op=AL.subtract
                )
                dsq = scrp.tile([P, NT], F32, tag="dsq")
                nc.scalar.activation(
                    out=dsq[:], in_=dd[:], func=AF.Square,
                    accum_out=d2col[:, s : s + 1],
                )

            pfin = pztp.tile([2, 1], F32, tag="pzt")
            nc.tensor.matmul(
                out=pfin[:], lhsT=d2col[:], rhs=ones_col[:], start=True, stop=True
            )
            osb = smallp.tile([2, 1], F32, tag="osb")
            nc.vector.tensor_copy(out=osb[:], in_=pfin[:])
            nc.sync.dma_start(out=out_part[:], in_=osb[:])

    with tile.TileContext(nc) as tc:
        _body(tc)
    if finalize:
        nc.finalize()
    return nc


def prep_inputs(z1, z2, batch_1, batch_2):
    """Pad/shard host-side and build all index-derived input tensors."""
    z1 = np.asarray(z1, dtype=np.float32)
    z2 = np.asarray(z2, dtype=np.float32)
    b1 = np.asarray(batch_1).astype(np.int64)
    b2 = np.asarray(batch_2).astype(np.int64)
    n = z1.shape[0]
    assert n <= NPAD, n

    def pad_z(z):
        out = np.zeros((NPAD, D), dtype=ml_dtypes.bfloat16)
        out[:n] = z.astype(ml_dtypes.bfloat16)
        return out

    def pad_b(b):
        out = np.full((NPAD,), G - 1, dtype=np.int64)
        out[:n] = b
        return out

    z1p, z2p, b1p, b2p = pad_z(z1), pad_z(z2), pad_b(b1), pad_b(b2)

    in_maps = []
    for c in range(NCORES):
        lo, hi = c * R, (c + 1) * R
        m = {"z1": z1p[lo:hi], "z2": z2p[lo:hi]}
        idx_cols = np.zeros((NT, 2, 2 * CK), dtype=np.int64)
        for s, b in ((0, b1p[lo:hi]), (1, b2p[lo:hi])):
            bt = b.reshape(NT, P)
            A = bt[:, 0]  # [NT]
            span = bt[:, -1] - bt[:, 0]
            assert span.max() <= CK - 1, (
                f"tile graph span {span.max()} exceeds CK-1; regenerate with larger CK"
            )
            # masks [P, NT*CK] interleaved
            mab = np.zeros((P, NT * CK), dtype=ml_dtypes.bfloat16)
            for j in range(CK):
                mab[:, j::CK] = (bt == (A + j)[:, None]).T.astype(ml_dtypes.bfloat16)
            m[f"mab{s + 1}"] = mab
            # scatter one-hot [2P, G]
            sel = np.zeros((2 * P, G), dtype=np.float32)
            crow = np.arange(NT * CK)
            gid = np.repeat(A, CK) + np.tile(np.arange(CK), NT)
            ok = gid < G
            sel[crow[ok], gid[ok]] = 1.0
            m[f"sel{s + 1}"] = sel
            # gather columns: for side s, order = [own@A.., other@A..]
            own, other = s, 1 - s
            idx_cols[:, s, :CK] = own * GW + (A[:, None] + np.arange(CK))
            idx_cols[:, s, CK:] = other * GW + (A[:, None] + np.arange(CK))
        flat = idx_cols.reshape(-1)  # [NIDX] in (t, s, q) order
        assert flat.max() < 2 * GW
        # indirect_copy wrapped layout: output col j reads the index at
        # partition (group*16 + j%16), free col j//16; same for every group.
        wrapped = np.zeros((P, NIDX // 16), dtype=np.uint16)
        for cg in range(8):
            wrapped[cg * 16 : (cg + 1) * 16, :] = flat.reshape(-1, 16).T
        m["gidx"] = wrapped
        in_maps.append(m)
    return in_maps


_NC_CACHE = {}


def _get_nc():
    if "nc" not in _NC_CACHE:
        _NC_CACHE["nc"] = build_nc()
    return _NC_CACHE["nc"]


def kernel(z1, z2, batch_1, batch_2):
    nc = _get_nc()
    in_maps = prep_inputs(z1, z2, batch_1, batch_2)
    res = run_bass_kernel_spmd(nc, in_maps, list(range(NCORES)))
    parts = np.stack([r["out_part"].reshape(2) for r in res.results])  # [8, 2]
    tot = parts.sum(axis=0)
    return np.float32(np.sqrt(tot[0]) + np.sqrt(tot[1]))

